# revision 18
# baseline (speedup 1.0000x reference)
"""Trainium2 Bass kernel for nn_DenseAttention (linear attention, no softmax).

Math (reassociated — the attention is fully linear, so the O(S^2) pre/attn
einsums collapse through a per-(b,q) Gram matrix):

    x  = hidden_states.reshape(b, t, s, h)
    G[b,q]    = x[b,:,q,:]^T @ x[b,:,q,:]                   # [h, h]
    Mf[b,a]   = sum_q qw[a,:,q,:] @ G[b,q] @ C[a, q*h:(q+1)*h, :]
    out[b,:,a*h:(a+1)*h] = x[b,:,a,:] @ Mf[b,a]

Sharding: 8 cores = (b in 0..1) x (a in 0..3). Each core streams x[b]
once for the Gram stage, computes its own Mf[b,a], and produces the
[2048, 256] output slice out[b, :, a*h:(a+1)*h]. Gather is concatenation.

Stage dtypes: the Gram stream (phase A) defaults to fp8-e4m3 using the PE
DoubleRow perf mode (contraction 256 deep per instruction, 2x row rate),
which halves both the x DMA bytes and the PE time of the dominant stage.
B/C/D stay bf16 (fp8 there fails the 2e-2 gate). Output is written bf16
and upcast on host.
"""

import os
import numpy as np
import ml_dtypes

import concourse.bass as bass
import concourse.mybir as mybir
import concourse.tile as tile
from concourse import bacc
from concourse.bass_utils import run_bass_kernel_spmd

BS, S, E = 2, 2048, 1024
SQ, H = 4, 256  # sqrt_n_heads, head_size
P = 128
NG = S // 256  # 8 row groups of 256 (2 k-subtiles of 128) for the fp8 stream
NT = S // P  # 16 row chunks of x

BF16 = mybir.dt.bfloat16
F32R = mybir.dt.float32r
F8 = mybir.dt.float8e4
DT = mybir.dt.float32

# Stage dtypes: A = Gram stream (x.T @ x), B = G @ C, C = qw @ T1, D = x @ Mf,
# O = output store dtype.
CFG = {
    "A": os.environ.get("KADT", "float8e4"),
    "B": os.environ.get("KBDT", "bfloat16"),
    "C": os.environ.get("KCDT", "bfloat16"),
    "D": os.environ.get("KDDT", "bfloat16"),
    "O": os.environ.get("KODT", "bfloat16"),
}

_PROGRAMS = {}
LAST_RESULTS = None  # test harness reads exec_time_ns from here


_NP_DT = {
    "bfloat16": ml_dtypes.bfloat16,
    "float8e4": ml_dtypes.float8_e4m3fn,
    "float32": np.float32,
    "float32r": np.float32,
}
_MM_DT = {
    "bfloat16": BF16,
    "float8e4": F8,
    "float32": DT,
    "float32r": F32R,
}


def _build_program(cfg):
    adt, bdt, cdt, ddt, odt = (_MM_DT[cfg[k]] for k in "ABCDO")
    fp8_stream = cfg["A"] == "float8e4"
    nc = bacc.Bacc("TRN2", target_bir_lowering=False, debug=False, num_devices=8)

    # Small operands arrive pre-packed in SBUF layout (partition-major) so
    # their DMAs are one contiguous run per partition.
    if fp8_stream:
        # xb[p, g, ks, e] = x[b, g*256 + ks*128 + p, e]
        xb = nc.dram_tensor("xb", [P, NG, 2, E], adt, kind="ExternalInput").ap()
    else:
        xb = nc.dram_tensor("xb", [S, E], adt, kind="ExternalInput").ap()
    xaT = nc.dram_tensor("xaT", [P, 2, S], ddt, kind="ExternalInput").ap()
    qwT = nc.dram_tensor("qwT", [P, SQ, 2, H], cdt, kind="ExternalInput").ap()
    cmb = nc.dram_tensor("cmb", [P, SQ, 2, H], bdt, kind="ExternalInput").ap()
    ident = nc.dram_tensor("ident", [P, P], BF16, kind="ExternalInput").ap()
    out = nc.dram_tensor("out", [S, H], odt, kind="ExternalOutput").ap()

    with tile.TileContext(nc) as tc:
        with (
            tc.tile_pool(name="xs", bufs=8) as xs_pool,
            tc.tile_pool(name="consts", bufs=1) as const_pool,
            tc.tile_pool(name="ps", bufs=8, space="PSUM") as ps_pool,
            tc.tile_pool(name="osb", bufs=1) as out_pool,
        ):
            xaT_sb = const_pool.tile([P, 2, S], ddt, tag="xaT")
            qwT_sb = const_pool.tile([P, SQ, 2, H], cdt, tag="qwT")
            c_sb = const_pool.tile([P, SQ, 2, H], bdt, tag="cmb")
            # g_sb[p, q, i, g] = G[q][i*128+p, g]; G is symmetric so the
            # partition axis can be read as either f or g.
            g_sb = const_pool.tile([P, SQ, 2, H], bdt, tag="gsb")
            ident_sb = const_pool.tile([P, P], BF16, tag="ident")

            # PSUM->SBUF copies alternate vector/scalar (gpsimd cannot read
            # PSUM on TRN2) so they never serialize behind one sequencer.
            _ci = [0]

            def cast(dst, src):
                i = _ci[0] % 2
                _ci[0] += 1
                if i == 0:
                    nc.vector.tensor_copy(dst, src)
                else:
                    nc.scalar.activation(
                        dst, src, mybir.ActivationFunctionType.Copy
                    )

            # PE p-state warmup: tiny matmuls on a memset tile keep the PE
            # busy from kernel start so the DVFS ramp overlaps the first DMA.
            warm_sb = const_pool.tile([P, 64], BF16, tag="warm")
            nc.gpsimd.memset(warm_sb[:], 0.0)
            warm_ps = ps_pool.tile([P, 32], DT, tag="ps", name="warm_ps")
            for _ in range(8):
                nc.tensor.matmul(
                    warm_ps[0:32, :], warm_sb[:, 0:32], warm_sb[:, 32:64],
                    start=True, stop=True,
                )

            # Phase A: G[q] (q=0..3) accumulated in PSUM over the t stream.
            # g_ps[q*2+fc][p, g] accumulates G[q][fc*128+p, g].
            g_ps = [
                ps_pool.tile([P, H if i % 2 == 0 else P], DT, tag="ps",
                             name=f"g_ps{i}")
                for i in range(8)
            ]

            if fp8_stream:
                # 256-row groups [p, ks, e]; DoubleRow contracts over (p, ks).
                # Group 0 loads q0's columns first so the PE starts early.
                for g in range(NG):
                    xt = xs_pool.tile([P, 2, E], adt, tag="xt", name=f"xt{g}")
                    if g == 0:
                        nc.sync.dma_start(
                            out=xt[:, :, 0:E // 2], in_=xb[:, 0, :, 0:E // 2]
                        )
                        nc.gpsimd.dma_start(
                            out=xt[:, :, E // 2:E], in_=xb[:, 0, :, E // 2:E]
                        )
                    else:
                        nc.sync.dma_start(out=xt[:], in_=xb[:, g])
                    for q in range(SQ):
                        # G symmetric: fc0 x full, fc1 x upper half only; the
                        # (1,0) block is transposed from (0,1) after the stream.
                        nc.tensor.matmul(
                            g_ps[q * 2][:],
                            xt[:, :, q * H: q * H + P],
                            xt[:, :, q * H:(q + 1) * H],
                            start=(g == 0),
                            stop=(g == NG - 1),
                            perf_mode=mybir.MatmulPerfMode.DoubleRow,
                        )
                        nc.tensor.matmul(
                            g_ps[q * 2 + 1][:],
                            xt[:, :, q * H + P: q * H + 2 * P],
                            xt[:, :, q * H + P: q * H + 2 * P],
                            start=(g == 0),
                            stop=(g == NG - 1),
                            perf_mode=mybir.MatmulPerfMode.DoubleRow,
                        )
            else:
                def g_mms(xt_c, ti):
                    for q in range(SQ):
                        for fc in range(2):
                            nc.tensor.matmul(
                                g_ps[q * 2 + fc][:],
                                xt_c[:, q * H + fc * P: q * H + fc * P + P],
                                xt_c[:, q * H:(q + 1) * H],
                                start=(ti == 0),
                                stop=(ti == NT - 1),
                            )

                for ti in range(2):
                    xt = xs_pool.tile([P, E], adt, tag="xt", name=f"xt{ti}")
                    if ti == 0:
                        nc.sync.dma_start(
                            out=xt[:, 0:E // 2], in_=xb[0:P, 0:E // 2]
                        )
                        nc.sync.dma_start(
                            out=xt[:, E // 2:E], in_=xb[0:P, E // 2:E]
                        )
                    else:
                        nc.sync.dma_start(out=xt[:], in_=xb[ti * P:(ti + 1) * P, :])
                    g_mms(xt[:], ti)
                for tp in range(1, NT // 2):
                    xt2 = xs_pool.tile([P, 2, E], adt, tag="xt2")
                    nc.sync.dma_start(
                        out=xt2[:],
                        in_=xb[tp * 2 * P:(tp + 1) * 2 * P, :].rearrange(
                            "(c p) e -> p c e", p=P
                        ),
                    )
                    for c in range(2):
                        g_mms(xt2[:, c], tp * 2 + c)

            # Consts queue on Sync after the x stream (so they cannot starve
            # it), in the order the compute phases need them.
            nc.sync.dma_start(out=c_sb[:], in_=cmb[:])
            nc.sync.dma_start(out=ident_sb[:], in_=ident[:])
            nc.sync.dma_start(out=qwT_sb[:], in_=qwT[:])
            nc.sync.dma_start(out=xaT_sb[:], in_=xaT[:])

            # Phases B+C, software-pipelined per q: B(q) fills T1[q] while
            # C(q-1) accumulates into Mf. mf_ps is allocated between the q=1
            # g-casts and t1 tiles so every PSUM slot-reuse dep points at an
            # already-emitted cast.
            t1_sb = const_pool.tile([P, SQ, 2, H], cdt, tag="t1")
            mf_sb = const_pool.tile([P, 2, H], ddt, tag="mf")
            mf_ps = [None, None]

            def phase_b(q):
                for fc in range(2):
                    t1_ps = ps_pool.tile([P, H], DT, tag="ps")
                    for gc in range(2):
                        nc.tensor.matmul(
                            t1_ps[:],
                            g_sb[:, q, gc, fc * P:(fc + 1) * P],
                            c_sb[:, q, gc, :],
                            start=(gc == 0),
                            stop=(gc == 1),
                        )
                    cast(t1_sb[:, q, fc, :], t1_ps[:])

            def phase_c(q):
                for ec in range(2):
                    for fc in range(2):
                        nc.tensor.matmul(
                            mf_ps[ec][:],
                            qwT_sb[:, q, fc, ec * P:(ec + 1) * P],
                            t1_sb[:, q, fc, :],
                            start=(q == 0 and fc == 0),
                            stop=(q == SQ - 1 and fc == 1),
                        )

            for q in range(SQ):
                cast(g_sb[:, q, 0, :], g_ps[q * 2][:])
                cast(g_sb[:, q, 1, P:H], g_ps[q * 2 + 1][:])
                tr_ps = ps_pool.tile([P, P], BF16, tag="ps", name=f"tr_ps{q}")
                nc.tensor.transpose(tr_ps[:], g_sb[:, q, 0, P:H], ident_sb[:])
                cast(g_sb[:, q, 1, 0:P], tr_ps[:])
                if q == 1:
                    mf_ps[0] = ps_pool.tile([P, H], DT, tag="ps", name="mf_ps0")
                    mf_ps[1] = ps_pool.tile([P, H], DT, tag="ps", name="mf_ps1")
                phase_b(q)
                if q >= 1:
                    phase_c(q - 1)
            phase_c(SQ - 1)
            for ec in range(2):
                cast(mf_sb[:, ec, :], mf_ps[ec][:])

            # Phase D: out rows = x[b,:,a,:] @ Mf. Results gather in one SBUF
            # buffer; 4 batched DMA issues (split across Sync/Scalar HWDGE).
            o_sb = out_pool.tile([P, NT, H], odt, tag="osb")
            for ti in range(NT):
                o_ps = ps_pool.tile([P, H], DT, tag="ps")
                for ec in range(2):
                    nc.tensor.matmul(
                        o_ps[:],
                        xaT_sb[:, ec, ti * P:(ti + 1) * P],
                        mf_sb[:, ec, :],
                        start=(ec == 0),
                        stop=(ec == 1),
                    )
                cast(o_sb[:, ti, :], o_ps[:])
                if ti % 2 == 1:
                    eng = nc.sync if (ti // 2) % 2 == 0 else nc.scalar
                    eng.dma_start(
                        out=out[(ti - 1) * P:(ti + 1) * P, :].rearrange(
                            "(c p) g -> p c g", p=P
                        ),
                        in_=o_sb[:, ti - 1:ti + 1, :],
                    )

    nc.compile()
    return nc


def _get_program(cfg=None):
    cfg = cfg or CFG
    key = tuple(cfg[k] for k in "ABCDO")
    if key not in _PROGRAMS:
        _PROGRAMS[key] = _build_program(cfg)
    return _PROGRAMS[key]


def _make_in_maps(hidden_states, queries, combiners, cfg=None):
    cfg = cfg or CFG
    adt, bdt, cdt, ddt = (_NP_DT[cfg[k]] for k in "ABCD")
    fp8_stream = cfg["A"] == "float8e4"
    x = np.ascontiguousarray(np.asarray(hidden_states, dtype=np.float32))
    qs = np.asarray(queries, dtype=np.float32)
    cb = np.asarray(combiners, dtype=np.float32)
    in_maps = []
    for c in range(8):
        b, a = divmod(c, 4)
        if fp8_stream:
            # xbp[p, g, ks, e] = x[b, g*256 + ks*128 + p, e]
            xbp = x[b].reshape(NG, 2, P, E).transpose(2, 0, 1, 3)
        else:
            xbp = x[b]
        # Layouts match the SBUF tiles exactly (partition dim first).
        # xaT[p, ec, t] = x[b, t, a*H + ec*128 + p]
        xaT = x[b][:, a * H:(a + 1) * H].T.reshape(2, P, S).transpose(1, 0, 2)
        # qwT[p, q, fc, e] = qw[a, e, q, fc*128+p]
        qwTp = qs[a].reshape(H, SQ, 2, P).transpose(3, 1, 2, 0)
        # cmb[p, q, gc, g2] = combiners[a, q*256 + gc*128 + p, g2]
        cmbp = cb[a].reshape(SQ, 2, P, H).transpose(2, 0, 1, 3)
        in_maps.append({
            "ident": np.eye(P, dtype=ml_dtypes.bfloat16),
            "xb": np.ascontiguousarray(xbp).astype(adt),
            "xaT": np.ascontiguousarray(xaT).astype(ddt),
            "qwT": np.ascontiguousarray(qwTp).astype(cdt),
            "cmb": np.ascontiguousarray(cmbp).astype(bdt),
        })
    return in_maps


def kernel(hidden_states, queries, combiners, cfg=None):
    global LAST_RESULTS
    cfg = cfg or CFG
    nc = _get_program(cfg)
    in_maps = _make_in_maps(hidden_states, queries, combiners, cfg)
    res = run_bass_kernel_spmd(
        nc, in_maps, core_ids=list(range(8)),
        trace=bool(os.environ.get("BASS_TRACE")),
    )
    LAST_RESULTS = res
    out = np.empty((BS, S, E), dtype=np.float32)
    for c in range(8):
        b, a = divmod(c, 4)
        out[b, :, a * H:(a + 1) * H] = res.results[c]["out"].astype(np.float32)
    return out


# revision 19
# speedup vs baseline: 1.0713x; 1.0713x over previous
"""Trainium2 Bass kernel for nn_DenseAttention (linear attention, no softmax).

Math (reassociated — the attention is fully linear, so the O(S^2) pre/attn
einsums collapse through a per-(b,q) Gram matrix):

    x  = hidden_states.reshape(b, t, s, h)
    G[b,q]    = x[b,:,q,:]^T @ x[b,:,q,:]                   # [h, h]
    Mf[b,a]   = sum_q qw[a,:,q,:] @ G[b,q] @ C[a, q*h:(q+1)*h, :]
    out[b,:,a*h:(a+1)*h] = x[b,:,a,:] @ Mf[b,a]

Sharding: 8 cores = (b in 0..1) x (a in 0..3). Each core streams x[b]
once for the Gram stage, computes its own Mf[b,a], and produces the
[2048, 256] output slice out[b, :, a*h:(a+1)*h]. Gather is concatenation.

Stage dtypes: the Gram stream (phase A) defaults to fp8-e4m3 using the PE
DoubleRow perf mode (contraction 256 deep per instruction, 2x row rate),
which halves both the x DMA bytes and the PE time of the dominant stage.
B/C/D stay bf16 (fp8 there fails the 2e-2 gate). Output is written bf16
and upcast on host.
"""

import os
import numpy as np
import ml_dtypes

import concourse.bass as bass
import concourse.mybir as mybir
import concourse.tile as tile
from concourse import bacc
from concourse.bass_utils import run_bass_kernel_spmd

BS, S, E = 2, 2048, 1024
SQ, H = 4, 256  # sqrt_n_heads, head_size
P = 128
NG = S // 256  # 8 row groups of 256 (2 k-subtiles of 128) for the fp8 stream
NT = S // P  # 16 row chunks of x

BF16 = mybir.dt.bfloat16
F32R = mybir.dt.float32r
F8 = mybir.dt.float8e4
DT = mybir.dt.float32

# Stage dtypes: A = Gram stream (x.T @ x), B = G @ C, C = qw @ T1, D = x @ Mf,
# O = output store dtype.
CFG = {
    "A": os.environ.get("KADT", "float8e4"),
    "B": os.environ.get("KBDT", "bfloat16"),
    "C": os.environ.get("KCDT", "bfloat16"),
    "D": os.environ.get("KDDT", "bfloat16"),
    "O": os.environ.get("KODT", "bfloat16"),
}

_PROGRAMS = {}
LAST_RESULTS = None  # test harness reads exec_time_ns from here


_NP_DT = {
    "bfloat16": ml_dtypes.bfloat16,
    "float8e4": ml_dtypes.float8_e4m3fn,
    "float32": np.float32,
    "float32r": np.float32,
}
_MM_DT = {
    "bfloat16": BF16,
    "float8e4": F8,
    "float32": DT,
    "float32r": F32R,
}


def _build_program(cfg):
    adt, bdt, cdt, ddt, odt = (_MM_DT[cfg[k]] for k in "ABCDO")
    fp8_stream = cfg["A"] == "float8e4"
    nc = bacc.Bacc("TRN2", target_bir_lowering=False, debug=False, num_devices=8)

    # Small operands arrive pre-packed in SBUF layout (partition-major) so
    # their DMAs are one contiguous run per partition.
    if fp8_stream:
        # xb[p, g, ks, e] = x[b, g*256 + ks*128 + p, e]
        xb = nc.dram_tensor("xb", [P, NG, 2, E], adt, kind="ExternalInput").ap()
    else:
        xb = nc.dram_tensor("xb", [S, E], adt, kind="ExternalInput").ap()
    xaT = nc.dram_tensor("xaT", [P, 2, S], ddt, kind="ExternalInput").ap()
    qwT = nc.dram_tensor("qwT", [P, SQ, 2, H], cdt, kind="ExternalInput").ap()
    cmb = nc.dram_tensor("cmb", [P, SQ, 2, H], bdt, kind="ExternalInput").ap()
    ident = nc.dram_tensor("ident", [P, P], BF16, kind="ExternalInput").ap()
    out = nc.dram_tensor("out", [S, H], odt, kind="ExternalOutput").ap()

    with tile.TileContext(nc) as tc:
        with (
            tc.tile_pool(name="xs", bufs=8) as xs_pool,
            tc.tile_pool(name="consts", bufs=1) as const_pool,
            tc.tile_pool(name="ps", bufs=8, space="PSUM") as ps_pool,
            tc.tile_pool(name="osb", bufs=1) as out_pool,
        ):
            xaT_sb = const_pool.tile([P, 2, S], ddt, tag="xaT")
            qwT_sb = const_pool.tile([P, SQ, 2, H], cdt, tag="qwT")
            c_sb = const_pool.tile([P, SQ, 2, H], bdt, tag="cmb")
            # g_sb[p, q, i, g] = G[q][i*128+p, g]; G is symmetric so the
            # partition axis can be read as either f or g.
            g_sb = const_pool.tile([P, SQ, 2, H], bdt, tag="gsb")
            ident_sb = const_pool.tile([P, P], BF16, tag="ident")

            # PSUM->SBUF copies alternate vector/scalar (gpsimd cannot read
            # PSUM on TRN2) so they never serialize behind one sequencer.
            _ci = [0]

            def cast(dst, src):
                i = _ci[0] % 2
                _ci[0] += 1
                if i == 0:
                    nc.vector.tensor_copy(dst, src)
                else:
                    nc.scalar.activation(
                        dst, src, mybir.ActivationFunctionType.Copy
                    )

            # PE p-state warmup: tiny matmuls on a memset tile keep the PE
            # busy from kernel start so the DVFS ramp overlaps the first DMA.
            warm_sb = const_pool.tile([P, 64], BF16, tag="warm")
            nc.gpsimd.memset(warm_sb[:], 0.0)
            warm_ps = ps_pool.tile([P, 32], DT, tag="ps", name="warm_ps")
            for _ in range(8):
                nc.tensor.matmul(
                    warm_ps[0:32, :], warm_sb[:, 0:32], warm_sb[:, 32:64],
                    start=True, stop=True,
                )

            # Phase A: G[q] (q=0..3) accumulated in PSUM over the t stream.
            # g_ps[q*2+fc][p, g] accumulates G[q][fc*128+p, g].
            g_ps = [
                ps_pool.tile([P, H if i % 2 == 0 else P], DT, tag="ps",
                             name=f"g_ps{i}")
                for i in range(8)
            ]

            if fp8_stream:
                # 256-row groups [p, ks, e]; DoubleRow contracts over (p, ks).
                # Group 0 loads q0's columns first so the PE starts early.
                for g in range(NG):
                    xt = xs_pool.tile([P, 2, E], adt, tag="xt", name=f"xt{g}")
                    if g == 0:
                        nc.sync.dma_start(
                            out=xt[:, :, 0:E // 2], in_=xb[:, 0, :, 0:E // 2]
                        )
                        nc.gpsimd.dma_start(
                            out=xt[:, :, E // 2:E], in_=xb[:, 0, :, E // 2:E]
                        )
                    else:
                        nc.sync.dma_start(out=xt[:], in_=xb[:, g])
                    for q in range(SQ):
                        # G symmetric: fc0 x full, fc1 x upper half only; the
                        # (1,0) block is transposed from (0,1) after the stream.
                        nc.tensor.matmul(
                            g_ps[q * 2][:],
                            xt[:, :, q * H: q * H + P],
                            xt[:, :, q * H:(q + 1) * H],
                            start=(g == 0),
                            stop=(g == NG - 1),
                            perf_mode=mybir.MatmulPerfMode.DoubleRow,
                        )
                        nc.tensor.matmul(
                            g_ps[q * 2 + 1][:],
                            xt[:, :, q * H + P: q * H + 2 * P],
                            xt[:, :, q * H + P: q * H + 2 * P],
                            start=(g == 0),
                            stop=(g == NG - 1),
                            perf_mode=mybir.MatmulPerfMode.DoubleRow,
                        )
            else:
                def g_mms(xt_c, ti):
                    for q in range(SQ):
                        for fc in range(2):
                            nc.tensor.matmul(
                                g_ps[q * 2 + fc][:],
                                xt_c[:, q * H + fc * P: q * H + fc * P + P],
                                xt_c[:, q * H:(q + 1) * H],
                                start=(ti == 0),
                                stop=(ti == NT - 1),
                            )

                for ti in range(2):
                    xt = xs_pool.tile([P, E], adt, tag="xt", name=f"xt{ti}")
                    if ti == 0:
                        nc.sync.dma_start(
                            out=xt[:, 0:E // 2], in_=xb[0:P, 0:E // 2]
                        )
                        nc.sync.dma_start(
                            out=xt[:, E // 2:E], in_=xb[0:P, E // 2:E]
                        )
                    else:
                        nc.sync.dma_start(out=xt[:], in_=xb[ti * P:(ti + 1) * P, :])
                    g_mms(xt[:], ti)
                for tp in range(1, NT // 2):
                    xt2 = xs_pool.tile([P, 2, E], adt, tag="xt2")
                    nc.sync.dma_start(
                        out=xt2[:],
                        in_=xb[tp * 2 * P:(tp + 1) * 2 * P, :].rearrange(
                            "(c p) e -> p c e", p=P
                        ),
                    )
                    for c in range(2):
                        g_mms(xt2[:, c], tp * 2 + c)

            # Consts queue on Sync after the x stream (so they cannot starve
            # it), in the order the compute phases need them.
            nc.sync.dma_start(out=c_sb[:], in_=cmb[:])
            nc.sync.dma_start(out=ident_sb[:], in_=ident[:])
            nc.sync.dma_start(out=qwT_sb[:], in_=qwT[:])
            nc.sync.dma_start(out=xaT_sb[:], in_=xaT[:])

            # Phases B+C, software-pipelined per q: B(q) fills T1[q] while
            # C(q-1) accumulates into Mf. mf_ps is allocated between the q=1
            # g-casts and t1 tiles so every PSUM slot-reuse dep points at an
            # already-emitted cast.
            t1_sb = const_pool.tile([P, SQ, 2, H], cdt, tag="t1")
            mf_sb = const_pool.tile([P, 2, H], ddt, tag="mf")
            mf_ps = [None, None]

            def phase_b(q):
                for fc in range(2):
                    t1_ps = ps_pool.tile([P, H], DT, tag="ps")
                    for gc in range(2):
                        nc.tensor.matmul(
                            t1_ps[:],
                            g_sb[:, q, gc, fc * P:(fc + 1) * P],
                            c_sb[:, q, gc, :],
                            start=(gc == 0),
                            stop=(gc == 1),
                        )
                    cast(t1_sb[:, q, fc, :], t1_ps[:])

            def phase_c(q):
                for ec in range(2):
                    for fc in range(2):
                        nc.tensor.matmul(
                            mf_ps[ec][:],
                            qwT_sb[:, q, fc, ec * P:(ec + 1) * P],
                            t1_sb[:, q, fc, :],
                            start=(q == 0 and fc == 0),
                            stop=(q == SQ - 1 and fc == 1),
                        )

            for q in range(SQ):
                cast(g_sb[:, q, 0, :], g_ps[q * 2][:])
                cast(g_sb[:, q, 1, P:H], g_ps[q * 2 + 1][:])
                tr_ps = ps_pool.tile([P, P], BF16, tag="ps", name=f"tr_ps{q}")
                nc.tensor.transpose(tr_ps[:], g_sb[:, q, 0, P:H], ident_sb[:])
                cast(g_sb[:, q, 1, 0:P], tr_ps[:])
                if q == 1:
                    mf_ps[0] = ps_pool.tile([P, H], DT, tag="ps", name="mf_ps0")
                    mf_ps[1] = ps_pool.tile([P, H], DT, tag="ps", name="mf_ps1")
                phase_b(q)
                if q >= 1:
                    phase_c(q - 1)
            phase_c(SQ - 1)
            for ec in range(2):
                cast(mf_sb[:, ec, :], mf_ps[ec][:])

            # Phase D: out rows = x[b,:,a,:] @ Mf. Results gather in one SBUF
            # buffer; 4 batched DMA issues (split across Sync/Scalar HWDGE).
            o_sb = out_pool.tile([P, NT, H], odt, tag="osb")
            for ti in range(NT):
                o_ps = ps_pool.tile([P, H], DT, tag="ps")
                for ec in range(2):
                    nc.tensor.matmul(
                        o_ps[:],
                        xaT_sb[:, ec, ti * P:(ti + 1) * P],
                        mf_sb[:, ec, :],
                        start=(ec == 0),
                        stop=(ec == 1),
                    )
                cast(o_sb[:, ti, :], o_ps[:])
                if ti % 4 == 3:
                    nc.sync.dma_start(
                        out=out[(ti - 3) * P:(ti + 1) * P, :].rearrange(
                            "(c p) g -> p c g", p=P
                        ),
                        in_=o_sb[:, ti - 3:ti + 1, :],
                    )

    nc.compile()
    return nc


def _get_program(cfg=None):
    cfg = cfg or CFG
    key = tuple(cfg[k] for k in "ABCDO")
    if key not in _PROGRAMS:
        _PROGRAMS[key] = _build_program(cfg)
    return _PROGRAMS[key]


def _make_in_maps(hidden_states, queries, combiners, cfg=None):
    cfg = cfg or CFG
    adt, bdt, cdt, ddt = (_NP_DT[cfg[k]] for k in "ABCD")
    fp8_stream = cfg["A"] == "float8e4"
    x = np.ascontiguousarray(np.asarray(hidden_states, dtype=np.float32))
    qs = np.asarray(queries, dtype=np.float32)
    cb = np.asarray(combiners, dtype=np.float32)
    in_maps = []
    for c in range(8):
        b, a = divmod(c, 4)
        if fp8_stream:
            # xbp[p, g, ks, e] = x[b, g*256 + ks*128 + p, e]
            xbp = x[b].reshape(NG, 2, P, E).transpose(2, 0, 1, 3)
        else:
            xbp = x[b]
        # Layouts match the SBUF tiles exactly (partition dim first).
        # xaT[p, ec, t] = x[b, t, a*H + ec*128 + p]
        xaT = x[b][:, a * H:(a + 1) * H].T.reshape(2, P, S).transpose(1, 0, 2)
        # qwT[p, q, fc, e] = qw[a, e, q, fc*128+p]
        qwTp = qs[a].reshape(H, SQ, 2, P).transpose(3, 1, 2, 0)
        # cmb[p, q, gc, g2] = combiners[a, q*256 + gc*128 + p, g2]
        cmbp = cb[a].reshape(SQ, 2, P, H).transpose(2, 0, 1, 3)
        in_maps.append({
            "ident": np.eye(P, dtype=ml_dtypes.bfloat16),
            "xb": np.ascontiguousarray(xbp).astype(adt),
            "xaT": np.ascontiguousarray(xaT).astype(ddt),
            "qwT": np.ascontiguousarray(qwTp).astype(cdt),
            "cmb": np.ascontiguousarray(cmbp).astype(bdt),
        })
    return in_maps


def kernel(hidden_states, queries, combiners, cfg=None):
    global LAST_RESULTS
    cfg = cfg or CFG
    nc = _get_program(cfg)
    in_maps = _make_in_maps(hidden_states, queries, combiners, cfg)
    res = run_bass_kernel_spmd(
        nc, in_maps, core_ids=list(range(8)),
        trace=bool(os.environ.get("BASS_TRACE")),
    )
    LAST_RESULTS = res
    out = np.empty((BS, S, E), dtype=np.float32)
    for c in range(8):
        b, a = divmod(c, 4)
        out[b, :, a * H:(a + 1) * H] = res.results[c]["out"].astype(np.float32)
    return out
